# revision 10
# baseline (speedup 1.0000x reference)
"""Trainium2 Bass kernel for a dense transformer block (QKV+gate proj, RoPE,
QK-RMSNorm, causal SDPA, output-RMSNorm + SiLU gate, output projection).

Sharding: tensor-parallel over heads across 8 NeuronCores (2 heads/core).
Wq/Wk/Wv/Wg split column-wise, attention fully local per core; the per-core
attention outputs are AllGathered and the output projection is computed with
Wo split column-wise (each core produces a 256-column slice of the output),
which replaces the row-parallel all-reduce with a much smaller all-gather.

All matmuls run as float32r (full fp32 storage, ~1 cycle/row on the PE).
"""

import os
import sys

for _p in ("/opt/trn_rl_repo", "/root/.axon_site/_ro/trn_rl_repo"):
    if os.path.isdir(_p) and _p not in sys.path:
        sys.path.insert(0, _p)

import numpy as np

import concourse.bass as bass
import concourse.mybir as mybir
from concourse import bacc
from concourse.bass_utils import run_bass_kernel_spmd
from concourse.tile import TileContext

B, T, HID = 2, 2048, 2048
H, D = 16, 128
NCORES = 8
HC = H // NCORES          # heads per core = 2
DC = HC * D               # 256 head-dims per core
BT = B * T                # 4096 tokens
KT = HID // 128           # 16 contraction tiles
EPS = 1e-5
SCALE = 1.0 / float(np.sqrt(D))
HALF_LN_D = 0.5 * float(np.log(D))  # ln(sqrt(128))
NEG = -3.0e38

F32 = mybir.dt.float32
F32R = mybir.dt.float32r
AF = mybir.ActivationFunctionType
ALU = mybir.AluOpType

LAST_EXEC_TIME_NS = None
_CACHED_NC = None


def _proj_sweep(nc, tc, xT_r, w_aps, post):
    """One sweep over xT computing 2 matrices (4 head-groups) in transposed
    layout: psum[dhead 128, tok 1024] accumulated over 16 k-tiles."""
    with tc.tile_pool(name="sweep_w", bufs=1) as wpool, \
         tc.tile_pool(name="sweep_x", bufs=2) as xpool, \
         tc.tile_pool(name="sweep_ps", bufs=1, space="PSUM") as pps, \
         tc.tile_pool(name="sweep_t", bufs=1) as tpool:
        wsb = []
        for mi, w_ap in enumerate(w_aps):
            w_t = wpool.tile([128, KT, DC], F32R, tag=f"w{mi}", name=f"w{mi}")
            nc.sync.dma_start(
                out=w_t,
                in_=w_ap.rearrange("(kt p) n -> p kt n", p=128).bitcast(F32R))
            wsb.append(w_t)
        for nbb in range(BT // 1024):
            ps = {}
            for mi in range(2):
                for m in range(HC):
                    ps[(mi, m)] = pps.tile([128, 1024], F32,
                                           tag=f"pp{mi}{m}", name=f"pp{mi}{m}")
            for k in range(KT):
                xk = xpool.tile([128, 1024], F32R, tag="xk", name="xk")
                nc.sync.dma_start(
                    out=xk,
                    in_=xT_r[:, k, nbb * 1024:(nbb + 1) * 1024].bitcast(F32R))
                for mi in range(2):
                    for m in range(HC):
                        lhsT = wsb[mi][:, k, m * 128:(m + 1) * 128]
                        for h2 in range(2):
                            nc.tensor.matmul(
                                ps[(mi, m)][:, h2 * 512:(h2 + 1) * 512],
                                lhsT, xk[:, h2 * 512:(h2 + 1) * 512],
                                start=(k == 0), stop=(k == KT - 1))
            for mi in range(2):
                for m in range(HC):
                    post[mi](ps[(mi, m)], m, nbb, tpool, pps,
                             f"pp{mi}{m}")


def _build_nc():
    nc = bacc.Bacc("TRN2", target_bir_lowering=False, debug=False,
                   num_devices=NCORES)

    xT = nc.dram_tensor("xT", [HID, BT], F32, kind="ExternalInput").ap()
    wq = nc.dram_tensor("wq", [HID, DC], F32, kind="ExternalInput").ap()
    wk = nc.dram_tensor("wk", [HID, DC], F32, kind="ExternalInput").ap()
    wv = nc.dram_tensor("wv", [HID, DC], F32, kind="ExternalInput").ap()
    wg = nc.dram_tensor("wg", [HID, DC], F32, kind="ExternalInput").ap()
    wo = nc.dram_tensor("wo", [HID, DC], F32, kind="ExternalInput").ap()
    cos2 = nc.dram_tensor("cos2", [128, BT], F32, kind="ExternalInput").ap()
    sin2 = nc.dram_tensor("sin2", [128, BT], F32, kind="ExternalInput").ap()
    negm = nc.dram_tensor("negm", [128, 128], F32, kind="ExternalInput").ap()
    ones_in = nc.dram_tensor("ones_in", [128, 128], F32, kind="ExternalInput").ap()
    ident_in = nc.dram_tensor("ident_in", [128, 128], F32, kind="ExternalInput").ap()
    qrw = nc.dram_tensor("qrw", [128, 1], F32, kind="ExternalInput").ap()
    krw = nc.dram_tensor("krw", [128, 1], F32, kind="ExternalInput").ap()

    outT = nc.dram_tensor("outT", [DC, BT], F32, kind="ExternalOutput").ap()
    ag_in = nc.dram_tensor("ag_in", [DC, BT], F32).ap()
    gdram = nc.dram_tensor("gdram", [DC, BT], F32).ap()
    yall = nc.dram_tensor("yall", [NCORES * DC, BT], F32,
                          addr_space="Shared").ap()

    xT_r = xT.rearrange("(kt p) t -> p kt t", p=128)
    yall_r = yall.rearrange("(kt p) t -> p kt t", p=128)

    with TileContext(nc) as tc:
        with tc.tile_pool(name="const", bufs=1) as const:
            ones_r = const.tile([128, 128], F32R)
            nc.sync.dma_start(out=ones_r, in_=ones_in.bitcast(F32R))
            epsb = const.tile([128, 1], F32)
            nc.vector.memset(epsb, float(D) * EPS)
            halfb = const.tile([128, 1], F32)
            nc.vector.memset(halfb, HALF_LN_D)

            with tc.tile_pool(name="persist", bufs=1) as persist:
                # final (rope+rms applied) qT/kT per head, f32r [d, b*t]
                qTf = [persist.tile([128, BT], F32R, tag=f"qTf{m}",
                                    name=f"qTf{m}") for m in range(HC)]
                kTf = [persist.tile([128, BT], F32R, tag=f"kTf{m}",
                                    name=f"kTf{m}") for m in range(HC)]

                # ---- sweep A: q, k (rope + rms fused into evacuation) ----
                with tc.tile_pool(name="ropec", bufs=1) as rp:
                    cos_sb = rp.tile([128, BT], F32)
                    nc.sync.dma_start(out=cos_sb, in_=cos2)
                    sin_sb = rp.tile([128, BT], F32)
                    nc.sync.dma_start(out=sin_sb, in_=sin2)
                    qrw_sb = rp.tile([128, 1], F32)
                    nc.sync.dma_start(out=qrw_sb, in_=qrw)
                    krw_sb = rp.tile([128, 1], F32)
                    nc.sync.dma_start(out=krw_sb, in_=krw)

                    def make_qk_post(dest, w_scalar):
                        def post(ps, m, nbb, tpool, pps, pstag):
                            c0, c1 = nbb * 1024, (nbb + 1) * 1024
                            stage = tpool.tile([128, 1024], F32, tag="stage",
                                               name="stage", bufs=1)
                            nc.vector.tensor_copy(stage, ps)
                            sq = tpool.tile([128, 1024], F32R, tag="sq",
                                            name="sq", bufs=1)
                            nc.vector.tensor_mul(sq, stage, stage)
                            ss = pps.tile([128, 1024], F32, tag=pstag, name="ss")
                            for h2 in range(2):
                                sl = slice(h2 * 512, (h2 + 1) * 512)
                                nc.tensor.matmul(ss[:, sl], ones_r, sq[:, sl],
                                                 start=True, stop=True)
                            lnw = tpool.tile([128, 1024], F32, tag="lnw",
                                             name="lnw")
                            nc.scalar.activation(out=lnw, in_=ss, func=AF.Ln,
                                                 bias=epsb)
                            fac = tpool.tile([128, 1024], F32, tag="fac",
                                             name="fac")
                            nc.scalar.activation(out=fac, in_=lnw, func=AF.Exp,
                                                 scale=-0.5, bias=halfb)
                            cc = cos_sb[:, c0:c1]
                            ssn = sin_sb[:, c0:c1]
                            sw = tpool.tile([128, 1024], F32, tag="sw",
                                            name="sw")
                            nc.sync.dma_start(out=sw[0:64, :],
                                              in_=stage[64:128, :])
                            nc.sync.dma_start(out=sw[64:128, :],
                                              in_=stage[0:64, :])
                            u = tpool.tile([128, 1024], F32, tag="t12",
                                           name="u")
                            w = tpool.tile([128, 1024], F32, tag="t34",
                                           name="w")
                            nc.vector.tensor_mul(u, stage, cc)
                            nc.gpsimd.tensor_mul(w, sw, ssn)
                            ro = tpool.tile([128, 1024], F32, tag="ro", name="ro")
                            nc.vector.tensor_sub(ro[0:64, :], u[0:64, :],
                                                 w[0:64, :])
                            nc.vector.tensor_add(ro[64:128, :], u[64:128, :],
                                                 w[64:128, :])
                            # dest = (ro * w[d]) * factor  (fused, f32r out)
                            nc.vector.scalar_tensor_tensor(
                                out=dest[m][:, c0:c1], in0=ro, scalar=w_scalar,
                                in1=fac, op0=ALU.mult, op1=ALU.mult)
                        return post

                    _proj_sweep(nc, tc, xT_r, [wq, wk],
                                [make_qk_post(qTf, qrw_sb),
                                 make_qk_post(kTf, krw_sb)])

                # ---- sweep B: v (transpose to [tok, d]), gate (spill) ----
                v_sb = [persist.tile([128, BT // 128, 128], F32R, tag=f"v{m}",
                                     name=f"v{m}") for m in range(HC)]
                with tc.tile_pool(name="identp", bufs=1) as ip:
                    ident = ip.tile([128, 128], F32)
                    nc.sync.dma_start(out=ident, in_=ident_in)

                    def v_post(ps, m, nbb, tpool, pps, pstag):
                        stage = tpool.tile([128, 1024], F32, tag="stage",
                                           name="stage", bufs=2)
                        nc.vector.tensor_copy(stage, ps)
                        for j in range(8):
                            tp = pps.tile([128, 128], F32, tag=pstag,
                                          name="tp")
                            nc.tensor.transpose(
                                tp, stage[:, j * 128:(j + 1) * 128], ident)
                            nc.vector.tensor_copy(v_sb[m][:, nbb * 8 + j, :],
                                                  tp)

                    def g_post(ps, m, nbb, tpool, pps, pstag):
                        gst = tpool.tile([128, 1024], F32, tag="gst",
                                         name="gst", bufs=2)
                        nc.scalar.copy(gst, ps)
                        nc.sync.dma_start(
                            out=gdram[m * 128:(m + 1) * 128,
                                      nbb * 1024:(nbb + 1) * 1024],
                            in_=gst)

                    _proj_sweep(nc, tc, xT_r, [wv, wg], [v_post, g_post])

                # ---- attention ----
                with tc.tile_pool(name="at_ps", bufs=1, space="PSUM") as pps, \
                     tc.tile_pool(name="at_ps2", bufs=2, space="PSUM") as pps2, \
                     tc.tile_pool(name="at_slab", bufs=1) as slab, \
                     tc.tile_pool(name="at_t", bufs=2) as tpool, \
                     tc.tile_pool(name="at_bh", bufs=1) as bhpool, \
                     tc.tile_pool(name="at_c", bufs=1) as acp:
                    negm_sb = acp.tile([128, 128], F32)
                    nc.sync.dma_start(out=negm_sb, in_=negm)
                    for m in range(HC):
                        for b in range(B):
                            t0 = b * T
                            ystash = bhpool.tile([128, T], F32, tag="bhC",
                                                 name="ystash")
                            wy = bhpool.tile([128, T], F32, tag="bhA",
                                             name="wy")
                            for qb in range(T // 512):
                                nk = 4 * (qb + 1)
                                es = []
                                for i in range(nk):
                                    stp = pps2.tile([128, 512], F32, tag="st",
                                                    name="stp")
                                    nc.tensor.matmul(
                                        stp,
                                        kTf[m][:, t0 + i * 128:
                                               t0 + (i + 1) * 128],
                                        qTf[m][:, t0 + qb * 512:
                                               t0 + (qb + 1) * 512],
                                        start=True, stop=True)
                                    e = slab.tile([128, 512], F32R,
                                                  tag=f"es{i}", name=f"es{i}")
                                    q_off = i * 128 - qb * 512
                                    if q_off >= 0:
                                        nc.vector.tensor_add(
                                            stp[:, q_off:q_off + 128],
                                            stp[:, q_off:q_off + 128],
                                            negm_sb)
                                        if q_off > 0:
                                            nc.gpsimd.memset(
                                                e[:, 0:q_off].bitcast(F32), 0.0)
                                        nc.scalar.activation(
                                            out=e[:, q_off:],
                                            in_=stp[:, q_off:],
                                            func=AF.Exp, scale=SCALE)
                                    else:
                                        nc.scalar.activation(
                                            out=e, in_=stp, func=AF.Exp,
                                            scale=SCALE)
                                    es.append(e)
                                ytp = pps2.tile([128, 512], F32, tag="yt",
                                                name="ytp")
                                ssp = pps2.tile([128, 512], F32, tag="sexp",
                                                name="ssp")
                                for i in range(nk):
                                    nc.tensor.matmul(
                                        ytp, v_sb[m][:, b * 16 + i, :], es[i],
                                        start=(i == 0), stop=(i == nk - 1))
                                    nc.tensor.matmul(
                                        ssp, ones_r, es[i],
                                        start=(i == 0), stop=(i == nk - 1))
                                qsl = slice(qb * 512, (qb + 1) * 512)
                                sq = tpool.tile([128, 512], F32R, tag="ysq",
                                                name="ysq")
                                nc.scalar.activation(out=sq, in_=ytp,
                                                     func=AF.Square)
                                nc.scalar.copy(ystash[:, qsl], ytp)
                                ssyp = pps.tile([128, 512], F32, tag="ssy",
                                                name="ssyp")
                                nc.tensor.matmul(ssyp, ones_r, sq,
                                                 start=True, stop=True)
                                s2t = tpool.tile([128, 512], F32, tag="s2",
                                                 name="s2t")
                                nc.scalar.activation(out=s2t, in_=ssp,
                                                     func=AF.Square)
                                # wy = ssy + (D*eps) * s^2
                                nc.vector.scalar_tensor_tensor(
                                    out=wy[:, qsl], in0=s2t,
                                    scalar=float(D) * EPS, in1=ssyp,
                                    op0=ALU.mult, op1=ALU.add)
                            # per-(b,h) tail: f = exp(-0.5 ln wy + ln(sqrt(D)))
                            fb = bhpool.tile([128, T], F32, tag="bhB",
                                             name="fb")
                            nc.scalar.activation(out=fb, in_=wy, func=AF.Ln)
                            fb2 = bhpool.tile([128, T], F32, tag="bhA",
                                              name="fb2")
                            nc.scalar.activation(out=fb2, in_=fb, func=AF.Exp,
                                                 scale=-0.5, bias=halfb)
                            gld = bhpool.tile([128, T], F32, tag="bhB",
                                              name="gld")
                            nc.sync.dma_start(
                                out=gld,
                                in_=gdram[m * 128:(m + 1) * 128, t0:t0 + T])
                            sg = bhpool.tile([128, T], F32, tag="bhE",
                                             name="sg")
                            nc.scalar.activation(out=sg, in_=gld, func=AF.Silu)
                            yf1 = bhpool.tile([128, T], F32, tag="bhD",
                                              name="yf1")
                            nc.vector.tensor_mul(yf1, ystash, fb2)
                            yf = bhpool.tile([128, T], F32, tag="bhA",
                                             name="yf")
                            nc.vector.tensor_mul(yf, yf1, sg)
                            nc.sync.dma_start(
                                out=ag_in[m * 128:(m + 1) * 128, t0:t0 + T],
                                in_=yf)

            nc.gpsimd.collective_compute(
                "AllGather", ALU.bypass,
                ins=[ag_in], outs=[yall],
                replica_groups=[list(range(NCORES))],
            )

            # ---- output projection (column slice of Wo) ----
            with tc.tile_pool(name="fin_w", bufs=1) as wpool, \
                 tc.tile_pool(name="fin_y", bufs=3) as ypool, \
                 tc.tile_pool(name="fin_o", bufs=2) as opool, \
                 tc.tile_pool(name="fin_ps", bufs=1, space="PSUM") as pps:
                wo_sb = wpool.tile([128, KT, DC], F32R, tag="wo")
                nc.sync.dma_start(
                    out=wo_sb,
                    in_=wo.rearrange("(kt p) n -> p kt n", p=128).bitcast(F32R))
                for th in range(2):
                    po = [pps.tile([128, 2048], F32, tag=f"po{m}",
                                   name=f"po{m}") for m in range(HC)]
                    for kd in range(KT):
                        ysl = ypool.tile([128, 2048], F32R, tag="ysl",
                                         name="ysl")
                        nc.sync.dma_start(
                            out=ysl,
                            in_=yall_r[:, kd,
                                       th * 2048:(th + 1) * 2048].bitcast(F32R))
                        for m in range(HC):
                            lhsT = wo_sb[:, kd, m * 128:(m + 1) * 128]
                            for tb in range(4):
                                nc.tensor.matmul(
                                    po[m][:, tb * 512:(tb + 1) * 512], lhsT,
                                    ysl[:, tb * 512:(tb + 1) * 512],
                                    start=(kd == 0), stop=(kd == KT - 1))
                    for m in range(HC):
                        ot = opool.tile([128, 2048], F32, tag="ot", name="ot")
                        nc.vector.tensor_copy(ot, po[m])
                        nc.sync.dma_start(
                            out=outT[m * 128:(m + 1) * 128,
                                     th * 2048:(th + 1) * 2048],
                            in_=ot)

    nc.compile()
    return nc


def _get_nc():
    global _CACHED_NC
    if _CACHED_NC is None:
        _CACHED_NC = _build_nc()
    return _CACHED_NC


def kernel(x, Wq, Wk, Wv, Wg, Wo, q_rms_w, k_rms_w, o_norm_w):
    global LAST_EXEC_TIME_NS
    x = np.asarray(x, dtype=np.float32)
    Wq = np.asarray(Wq, dtype=np.float32)
    Wk = np.asarray(Wk, dtype=np.float32)
    Wv = np.asarray(Wv, dtype=np.float32)
    Wg = np.asarray(Wg, dtype=np.float32)
    Wo = np.asarray(Wo, dtype=np.float32)
    q_rms_w = np.asarray(q_rms_w, dtype=np.float32)
    k_rms_w = np.asarray(k_rms_w, dtype=np.float32)
    o_norm_w = np.asarray(o_norm_w, dtype=np.float32)

    xT = np.ascontiguousarray(x.reshape(BT, HID).T)
    # fold o_norm_w into Wo rows: (y*o_w) @ Wo == y @ (o_w[:,None]*Wo)
    wo_scaled = Wo * np.tile(o_norm_w, H)[:, None]

    inv = 1.0 / (10000.0 ** (np.arange(0, D, 2, dtype=np.float64) / D))
    pos = np.arange(T, dtype=np.float64)
    fr = pos[:, None] * inv[None, :]          # [T, 64]
    cosT = np.cos(fr).T.astype(np.float32)    # [64, T]
    sinT = np.sin(fr).T.astype(np.float32)
    cosbt = np.concatenate([cosT] * B, axis=1)
    sinbt = np.concatenate([sinT] * B, axis=1)
    cos2 = np.ascontiguousarray(np.vstack([cosbt, cosbt]))   # [128, BT]
    sin2 = np.ascontiguousarray(np.vstack([sinbt, sinbt]))

    kk, qq = np.meshgrid(np.arange(128), np.arange(128), indexing="ij")
    negm = np.where(kk <= qq, 0.0, NEG).astype(np.float32)
    ones128 = np.ones((128, 128), dtype=np.float32)
    ident = np.eye(128, dtype=np.float32)

    in_maps = []
    for c in range(NCORES):
        csl = slice(c * DC, (c + 1) * DC)
        in_maps.append({
            "xT": xT,
            "wq": np.ascontiguousarray(Wq[:, csl]),
            "wk": np.ascontiguousarray(Wk[:, csl]),
            "wv": np.ascontiguousarray(Wv[:, csl]),
            "wg": np.ascontiguousarray(Wg[:, csl]),
            "wo": np.ascontiguousarray(wo_scaled[:, csl]),
            "cos2": cos2,
            "sin2": sin2,
            "negm": negm,
            "ones_in": ones128,
            "ident_in": ident,
            "qrw": np.ascontiguousarray(q_rms_w.reshape(128, 1)),
            "krw": np.ascontiguousarray(k_rms_w.reshape(128, 1)),
        })

    nc = _get_nc()
    trace = os.environ.get("KERNEL_TRACE", "0") == "1"
    res = run_bass_kernel_spmd(nc, in_maps, list(range(NCORES)), trace=trace)
    LAST_EXEC_TIME_NS = res.exec_time_ns

    outT_full = np.concatenate([res.results[c]["outT"] for c in range(NCORES)],
                               axis=0)              # [2048 n, 4096 t]
    out = outT_full.T.reshape(B, T, HID)
    return np.ascontiguousarray(out)


# revision 11
# speedup vs baseline: 1.0305x; 1.0305x over previous
"""Trainium2 Bass kernel for a dense transformer block (QKV+gate proj, RoPE,
QK-RMSNorm, causal SDPA, output-RMSNorm + SiLU gate, output projection).

Sharding: tensor-parallel over heads across 8 NeuronCores (2 heads/core).
Wq/Wk/Wv/Wg split column-wise, attention fully local per core; the per-core
attention outputs are AllGathered and the output projection is computed with
Wo split column-wise (each core produces a 256-column slice of the output),
which replaces the row-parallel all-reduce with a much smaller all-gather.

All matmuls run as float32r (full fp32 storage, ~1 cycle/row on the PE).
"""

import os
import sys

for _p in ("/opt/trn_rl_repo", "/root/.axon_site/_ro/trn_rl_repo"):
    if os.path.isdir(_p) and _p not in sys.path:
        sys.path.insert(0, _p)

import numpy as np

import concourse.bass as bass
import concourse.mybir as mybir
from concourse import bacc
from concourse.bass_utils import run_bass_kernel_spmd
from concourse.tile import TileContext

B, T, HID = 2, 2048, 2048
H, D = 16, 128
NCORES = 8
HC = H // NCORES          # heads per core = 2
DC = HC * D               # 256 head-dims per core
BT = B * T                # 4096 tokens
KT = HID // 128           # 16 contraction tiles
EPS = 1e-5
SCALE = 1.0 / float(np.sqrt(D))
HALF_LN_D = 0.5 * float(np.log(D))  # ln(sqrt(128))
NEG = -3.0e38

F32 = mybir.dt.float32
F32R = mybir.dt.float32r
AF = mybir.ActivationFunctionType
ALU = mybir.AluOpType

LAST_EXEC_TIME_NS = None
_CACHED_NC = None


def _proj_sweep(nc, tc, xT_r, w_aps, post):
    """One sweep over xT computing 2 matrices (4 head-groups) in transposed
    layout: psum[dhead 128, tok 1024] accumulated over 16 k-tiles."""
    with tc.tile_pool(name="sweep_w", bufs=1) as wpool, \
         tc.tile_pool(name="sweep_x", bufs=2) as xpool, \
         tc.tile_pool(name="sweep_ps", bufs=1, space="PSUM") as pps, \
         tc.tile_pool(name="sweep_t", bufs=1) as tpool:
        wsb = []
        for mi, w_ap in enumerate(w_aps):
            w_t = wpool.tile([128, KT, DC], F32R, tag=f"w{mi}", name=f"w{mi}")
            nc.sync.dma_start(out=w_t, in_=w_ap.bitcast(F32R))
            wsb.append(w_t)
        for nbb in range(BT // 1024):
            ps = {}
            for mi in range(2):
                for m in range(HC):
                    ps[(mi, m)] = pps.tile([128, 1024], F32,
                                           tag=f"pp{mi}{m}", name=f"pp{mi}{m}")
            for k in range(KT):
                xk = xpool.tile([128, 1024], F32R, tag="xk", name="xk")
                nc.sync.dma_start(out=xk,
                                  in_=xT_r[k, nbb].bitcast(F32R))
                for mi in range(2):
                    for m in range(HC):
                        lhsT = wsb[mi][:, k, m * 128:(m + 1) * 128]
                        for h2 in range(2):
                            nc.tensor.matmul(
                                ps[(mi, m)][:, h2 * 512:(h2 + 1) * 512],
                                lhsT, xk[:, h2 * 512:(h2 + 1) * 512],
                                start=(k == 0), stop=(k == KT - 1))
            for mi in range(2):
                for m in range(HC):
                    post[mi](ps[(mi, m)], m, nbb, tpool, pps,
                             f"pp{mi}{m}")


def _build_nc():
    nc = bacc.Bacc("TRN2", target_bir_lowering=False, debug=False,
                   num_devices=NCORES)

    xT = nc.dram_tensor("xT", [KT, BT // 1024, 128, 1024], F32,
                        kind="ExternalInput").ap()
    wq = nc.dram_tensor("wq", [128, KT, DC], F32, kind="ExternalInput").ap()
    wk = nc.dram_tensor("wk", [128, KT, DC], F32, kind="ExternalInput").ap()
    wv = nc.dram_tensor("wv", [128, KT, DC], F32, kind="ExternalInput").ap()
    wg = nc.dram_tensor("wg", [128, KT, DC], F32, kind="ExternalInput").ap()
    wo = nc.dram_tensor("wo", [128, KT, DC], F32, kind="ExternalInput").ap()
    cos2 = nc.dram_tensor("cos2", [128, BT], F32, kind="ExternalInput").ap()
    sin2 = nc.dram_tensor("sin2", [128, BT], F32, kind="ExternalInput").ap()
    negm = nc.dram_tensor("negm", [128, 128], F32, kind="ExternalInput").ap()
    ones_in = nc.dram_tensor("ones_in", [128, 128], F32, kind="ExternalInput").ap()
    ident_in = nc.dram_tensor("ident_in", [128, 128], F32, kind="ExternalInput").ap()
    qrw = nc.dram_tensor("qrw", [128, 1], F32, kind="ExternalInput").ap()
    krw = nc.dram_tensor("krw", [128, 1], F32, kind="ExternalInput").ap()

    outT = nc.dram_tensor("outT", [DC, BT], F32, kind="ExternalOutput").ap()
    ag_in = [nc.dram_tensor(f"ag_in{b}", [DC, T], F32).ap() for b in range(B)]
    gdram = nc.dram_tensor("gdram", [DC, BT], F32).ap()
    yall = [nc.dram_tensor(f"yall{b}", [NCORES * DC, T], F32,
                           addr_space="Shared").ap() for b in range(B)]

    xT_r = xT

    with TileContext(nc) as tc:
        with tc.tile_pool(name="const", bufs=1) as const:
            ones_r = const.tile([128, 128], F32R)
            nc.sync.dma_start(out=ones_r, in_=ones_in.bitcast(F32R))
            epsb = const.tile([128, 1], F32)
            nc.vector.memset(epsb, float(D) * EPS)
            halfb = const.tile([128, 1], F32)
            nc.vector.memset(halfb, HALF_LN_D)

            with tc.tile_pool(name="persist", bufs=1) as persist:
                # final (rope+rms applied) qT/kT per head, f32r [d, b*t]
                qTf = [persist.tile([128, BT], F32R, tag=f"qTf{m}",
                                    name=f"qTf{m}") for m in range(HC)]
                kTf = [persist.tile([128, BT], F32R, tag=f"kTf{m}",
                                    name=f"kTf{m}") for m in range(HC)]

                # ---- sweep A: q, k (rope + rms fused into evacuation) ----
                with tc.tile_pool(name="ropec", bufs=1) as rp:
                    cos_sb = rp.tile([128, BT], F32)
                    nc.sync.dma_start(out=cos_sb, in_=cos2)
                    sin_sb = rp.tile([128, BT], F32)
                    nc.sync.dma_start(out=sin_sb, in_=sin2)
                    qrw_sb = rp.tile([128, 1], F32)
                    nc.sync.dma_start(out=qrw_sb, in_=qrw)
                    krw_sb = rp.tile([128, 1], F32)
                    nc.sync.dma_start(out=krw_sb, in_=krw)

                    def make_qk_post(dest, w_scalar):
                        def post(ps, m, nbb, tpool, pps, pstag):
                            c0, c1 = nbb * 1024, (nbb + 1) * 1024
                            stage = tpool.tile([128, 1024], F32, tag="stage",
                                               name="stage", bufs=1)
                            nc.vector.tensor_copy(stage, ps)
                            sq = tpool.tile([128, 1024], F32R, tag="sq",
                                            name="sq", bufs=1)
                            nc.vector.tensor_mul(sq, stage, stage)
                            ss = pps.tile([128, 1024], F32, tag=pstag, name="ss")
                            for h2 in range(2):
                                sl = slice(h2 * 512, (h2 + 1) * 512)
                                nc.tensor.matmul(ss[:, sl], ones_r, sq[:, sl],
                                                 start=True, stop=True)
                            lnw = tpool.tile([128, 1024], F32, tag="lnw",
                                             name="lnw")
                            nc.scalar.activation(out=lnw, in_=ss, func=AF.Ln,
                                                 bias=epsb)
                            fac = tpool.tile([128, 1024], F32, tag="fac",
                                             name="fac")
                            nc.scalar.activation(out=fac, in_=lnw, func=AF.Exp,
                                                 scale=-0.5, bias=halfb)
                            cc = cos_sb[:, c0:c1]
                            ssn = sin_sb[:, c0:c1]
                            sw = tpool.tile([128, 1024], F32, tag="sw",
                                            name="sw")
                            nc.sync.dma_start(out=sw[0:64, :],
                                              in_=stage[64:128, :])
                            nc.sync.dma_start(out=sw[64:128, :],
                                              in_=stage[0:64, :])
                            u = tpool.tile([128, 1024], F32, tag="t12",
                                           name="u")
                            w = tpool.tile([128, 1024], F32, tag="t34",
                                           name="w")
                            nc.vector.tensor_mul(u, stage, cc)
                            nc.gpsimd.tensor_mul(w, sw, ssn)
                            ro = tpool.tile([128, 1024], F32, tag="ro", name="ro")
                            nc.vector.tensor_sub(ro[0:64, :], u[0:64, :],
                                                 w[0:64, :])
                            nc.vector.tensor_add(ro[64:128, :], u[64:128, :],
                                                 w[64:128, :])
                            # dest = (ro * w[d]) * factor  (fused, f32r out)
                            nc.vector.scalar_tensor_tensor(
                                out=dest[m][:, c0:c1], in0=ro, scalar=w_scalar,
                                in1=fac, op0=ALU.mult, op1=ALU.mult)
                        return post

                    _proj_sweep(nc, tc, xT_r, [wq, wk],
                                [make_qk_post(qTf, qrw_sb),
                                 make_qk_post(kTf, krw_sb)])

                # ---- sweep B: v (transpose to [tok, d]), gate (spill) ----
                v_sb = [persist.tile([128, BT // 128, 128], F32R, tag=f"v{m}",
                                     name=f"v{m}") for m in range(HC)]
                with tc.tile_pool(name="identp", bufs=1) as ip:
                    ident = ip.tile([128, 128], F32)
                    nc.sync.dma_start(out=ident, in_=ident_in)

                    def v_post(ps, m, nbb, tpool, pps, pstag):
                        stage = tpool.tile([128, 1024], F32, tag="stage",
                                           name="stage", bufs=2)
                        nc.vector.tensor_copy(stage, ps)
                        for j in range(8):
                            tp = pps.tile([128, 128], F32, tag=pstag,
                                          name="tp")
                            nc.tensor.transpose(
                                tp, stage[:, j * 128:(j + 1) * 128], ident)
                            nc.vector.tensor_copy(v_sb[m][:, nbb * 8 + j, :],
                                                  tp)

                    def g_post(ps, m, nbb, tpool, pps, pstag):
                        gst = tpool.tile([128, 1024], F32, tag="gst",
                                         name="gst", bufs=2)
                        nc.scalar.copy(gst, ps)
                        nc.sync.dma_start(
                            out=gdram[m * 128:(m + 1) * 128,
                                      nbb * 1024:(nbb + 1) * 1024],
                            in_=gst)

                    _proj_sweep(nc, tc, xT_r, [wv, wg], [v_post, g_post])

                # ---- attention ----
                with tc.tile_pool(name="at_ps", bufs=1, space="PSUM") as pps, \
                     tc.tile_pool(name="at_ps2", bufs=2, space="PSUM") as pps2, \
                     tc.tile_pool(name="at_slab", bufs=1) as slab, \
                     tc.tile_pool(name="at_t", bufs=2) as tpool, \
                     tc.tile_pool(name="at_bh", bufs=1) as bhpool, \
                     tc.tile_pool(name="at_c", bufs=1) as acp:
                    negm_sb = acp.tile([128, 128], F32)
                    nc.sync.dma_start(out=negm_sb, in_=negm)
                    for b in range(B):
                      for m in range(HC):
                            t0 = b * T
                            ystash = bhpool.tile([128, T], F32, tag="bhC",
                                                 name="ystash")
                            wy = bhpool.tile([128, T], F32, tag="bhA",
                                             name="wy")
                            for qb in range(T // 512):
                                nk = 4 * (qb + 1)
                                es = []
                                for i in range(nk):
                                    stp = pps2.tile([128, 512], F32, tag="st",
                                                    name="stp")
                                    nc.tensor.matmul(
                                        stp,
                                        kTf[m][:, t0 + i * 128:
                                               t0 + (i + 1) * 128],
                                        qTf[m][:, t0 + qb * 512:
                                               t0 + (qb + 1) * 512],
                                        start=True, stop=True)
                                    e = slab.tile([128, 512], F32R,
                                                  tag=f"es{i}", name=f"es{i}")
                                    q_off = i * 128 - qb * 512
                                    if q_off >= 0:
                                        nc.vector.tensor_add(
                                            stp[:, q_off:q_off + 128],
                                            stp[:, q_off:q_off + 128],
                                            negm_sb)
                                        if q_off > 0:
                                            nc.gpsimd.memset(
                                                e[:, 0:q_off].bitcast(F32), 0.0)
                                        nc.scalar.activation(
                                            out=e[:, q_off:],
                                            in_=stp[:, q_off:],
                                            func=AF.Exp, scale=SCALE)
                                    else:
                                        nc.scalar.activation(
                                            out=e, in_=stp, func=AF.Exp,
                                            scale=SCALE)
                                    es.append(e)
                                ytp = pps2.tile([128, 512], F32, tag="yt",
                                                name="ytp")
                                ssp = pps2.tile([128, 512], F32, tag="sexp",
                                                name="ssp")
                                for i in range(nk):
                                    nc.tensor.matmul(
                                        ytp, v_sb[m][:, b * 16 + i, :], es[i],
                                        start=(i == 0), stop=(i == nk - 1))
                                    nc.tensor.matmul(
                                        ssp, ones_r, es[i],
                                        start=(i == 0), stop=(i == nk - 1))
                                qsl = slice(qb * 512, (qb + 1) * 512)
                                sq = tpool.tile([128, 512], F32R, tag="ysq",
                                                name="ysq")
                                nc.scalar.activation(out=sq, in_=ytp,
                                                     func=AF.Square)
                                nc.scalar.copy(ystash[:, qsl], ytp)
                                ssyp = pps.tile([128, 512], F32, tag="ssy",
                                                name="ssyp")
                                nc.tensor.matmul(ssyp, ones_r, sq,
                                                 start=True, stop=True)
                                s2t = tpool.tile([128, 512], F32, tag="s2",
                                                 name="s2t")
                                nc.scalar.activation(out=s2t, in_=ssp,
                                                     func=AF.Square)
                                # wy = ssy + (D*eps) * s^2
                                nc.vector.scalar_tensor_tensor(
                                    out=wy[:, qsl], in0=s2t,
                                    scalar=float(D) * EPS, in1=ssyp,
                                    op0=ALU.mult, op1=ALU.add)
                            # per-(b,h) tail: f = exp(-0.5 ln wy + ln(sqrt(D)))
                            fb = bhpool.tile([128, T], F32, tag="bhB",
                                             name="fb")
                            nc.scalar.activation(out=fb, in_=wy, func=AF.Ln)
                            fb2 = bhpool.tile([128, T], F32, tag="bhA",
                                              name="fb2")
                            nc.scalar.activation(out=fb2, in_=fb, func=AF.Exp,
                                                 scale=-0.5, bias=halfb)
                            gld = bhpool.tile([128, T], F32, tag="bhB",
                                              name="gld")
                            nc.sync.dma_start(
                                out=gld,
                                in_=gdram[m * 128:(m + 1) * 128, t0:t0 + T])
                            sg = bhpool.tile([128, T], F32, tag="bhE",
                                             name="sg")
                            nc.scalar.activation(out=sg, in_=gld, func=AF.Silu)
                            yf1 = bhpool.tile([128, T], F32, tag="bhD",
                                              name="yf1")
                            nc.vector.tensor_mul(yf1, ystash, fb2)
                            yf = bhpool.tile([128, T], F32, tag="bhA",
                                             name="yf")
                            nc.vector.tensor_mul(yf, yf1, sg)
                            nc.sync.dma_start(
                                out=ag_in[b][m * 128:(m + 1) * 128, :],
                                in_=yf)
                      nc.gpsimd.collective_compute(
                          "AllGather", ALU.bypass,
                          ins=[ag_in[b]], outs=[yall[b]],
                          replica_groups=[list(range(NCORES))],
                      )


            # ---- output projection (column slice of Wo) ----
            with tc.tile_pool(name="fin_w", bufs=1) as wpool, \
                 tc.tile_pool(name="fin_y", bufs=3) as ypool, \
                 tc.tile_pool(name="fin_o", bufs=2) as opool, \
                 tc.tile_pool(name="fin_ps", bufs=1, space="PSUM") as pps:
                wo_sb = wpool.tile([128, KT, DC], F32R, tag="wo")
                nc.sync.dma_start(out=wo_sb, in_=wo.bitcast(F32R))
                for b in range(B):
                    yall_b = yall[b].rearrange("(kt p) t -> p kt t", p=128)
                    for m in range(HC):
                        po = pps.tile([128, 2048], F32, tag=f"po{m}",
                                      name=f"po{m}")
                        for kd in range(KT):
                            ysl = ypool.tile([128, 2048], F32R,
                                             tag=f"ysl{m}", name="ysl")
                            nc.sync.dma_start(
                                out=ysl, in_=yall_b[:, kd, :].bitcast(F32R))
                            lhsT = wo_sb[:, kd, m * 128:(m + 1) * 128]
                            for tb in range(4):
                                nc.tensor.matmul(
                                    po[:, tb * 512:(tb + 1) * 512], lhsT,
                                    ysl[:, tb * 512:(tb + 1) * 512],
                                    start=(kd == 0), stop=(kd == KT - 1))
                        ot = opool.tile([128, 2048], F32, tag="ot", name="ot")
                        nc.vector.tensor_copy(ot, po)
                        nc.sync.dma_start(
                            out=outT[m * 128:(m + 1) * 128, b * T:(b + 1) * T],
                            in_=ot)
    nc.compile()
    return nc


def _get_nc():
    global _CACHED_NC
    if _CACHED_NC is None:
        _CACHED_NC = _build_nc()
    return _CACHED_NC


def kernel(x, Wq, Wk, Wv, Wg, Wo, q_rms_w, k_rms_w, o_norm_w):
    global LAST_EXEC_TIME_NS
    x = np.asarray(x, dtype=np.float32)
    Wq = np.asarray(Wq, dtype=np.float32)
    Wk = np.asarray(Wk, dtype=np.float32)
    Wv = np.asarray(Wv, dtype=np.float32)
    Wg = np.asarray(Wg, dtype=np.float32)
    Wo = np.asarray(Wo, dtype=np.float32)
    q_rms_w = np.asarray(q_rms_w, dtype=np.float32)
    k_rms_w = np.asarray(k_rms_w, dtype=np.float32)
    o_norm_w = np.asarray(o_norm_w, dtype=np.float32)

    xT = x.reshape(BT, HID).T          # [HID, BT]
    # [KT, BT//1024, 128, 1024] contiguous chunks
    xt4 = np.ascontiguousarray(
        xT.reshape(KT, 128, BT // 1024, 1024).transpose(0, 2, 1, 3))
    # fold o_norm_w into Wo rows: (y*o_w) @ Wo == y @ (o_w[:,None]*Wo)
    wo_scaled = Wo * np.tile(o_norm_w, H)[:, None]

    inv = 1.0 / (10000.0 ** (np.arange(0, D, 2, dtype=np.float64) / D))
    pos = np.arange(T, dtype=np.float64)
    fr = pos[:, None] * inv[None, :]          # [T, 64]
    cosT = np.cos(fr).T.astype(np.float32)    # [64, T]
    sinT = np.sin(fr).T.astype(np.float32)
    cosbt = np.concatenate([cosT] * B, axis=1)
    sinbt = np.concatenate([sinT] * B, axis=1)
    cos2 = np.ascontiguousarray(np.vstack([cosbt, cosbt]))   # [128, BT]
    sin2 = np.ascontiguousarray(np.vstack([sinbt, sinbt]))

    kk, qq = np.meshgrid(np.arange(128), np.arange(128), indexing="ij")
    negm = np.where(kk <= qq, 0.0, NEG).astype(np.float32)
    ones128 = np.ones((128, 128), dtype=np.float32)
    ident = np.eye(128, dtype=np.float32)

    in_maps = []
    for c in range(NCORES):
        csl = slice(c * DC, (c + 1) * DC)
        def wt(wmat):
            # [HID, DC] -> [128, KT, DC] matching the SBUF tile layout
            return np.ascontiguousarray(
                wmat[:, csl].reshape(KT, 128, DC).transpose(1, 0, 2))
        in_maps.append({
            "xT": xt4,
            "wq": wt(Wq),
            "wk": wt(Wk),
            "wv": wt(Wv),
            "wg": wt(Wg),
            "wo": wt(wo_scaled),
            "cos2": cos2,
            "sin2": sin2,
            "negm": negm,
            "ones_in": ones128,
            "ident_in": ident,
            "qrw": np.ascontiguousarray(q_rms_w.reshape(128, 1)),
            "krw": np.ascontiguousarray(k_rms_w.reshape(128, 1)),
        })

    nc = _get_nc()
    trace = os.environ.get("KERNEL_TRACE", "0") == "1"
    res = run_bass_kernel_spmd(nc, in_maps, list(range(NCORES)), trace=trace)
    LAST_EXEC_TIME_NS = res.exec_time_ns

    outT_full = np.concatenate([res.results[c]["outT"] for c in range(NCORES)],
                               axis=0)              # [2048 n, 4096 t]
    out = outT_full.T.reshape(B, T, HID)
    return np.ascontiguousarray(out)
